# revision 23
# baseline (speedup 1.0000x reference)
"""GroupedQueryAttention Trainium2 kernel (8 NeuronCores).

Sharding: (batch b in 0..1) x (kv-head group g in 0..3) -> core 4*b+g.
Each core computes, for its batch, the 4 query heads (4g..4g+3) that share
kv head g, plus the partial output projection through the matching 512-row
slice of Wo.  The host sums the 4 bf16 partials per batch in f32.

On-device dataflow is fully "transposed": activations live as [feature,
token] so every matmul contraction sits on the partition axis, and the
softmax probabilities come out directly in the layout the P@V matmul
needs.  Performance structure vs the phase-serial baseline:
  - host pre-arranges every DRAM operand so each DMA is contiguous per
    partition; x arrives in per-chunk pieces on the sync queue while the
    weight tensors stream on the scalar queue
  - dummy warm-up matmuls run during the initial DMA wait so the PE HAM
    clock gate is at 8/8 when real work starts
  - causal diagonal 512-blocks are computed on restricted query ranges
    (512/384/256/128 wide) instead of full width + mask
  - softmax denominators are accumulated on the Vector engine (bf16 adds
    over the probability tiles) with a single ones-matmul partition
    reduction per (chunk, head); 1/den uses reciprocal_approx_fast
  - rmsnorm uses reciprocal_approx_fast + a Sqrt activation, keeping the
    Scalar engine on two activation-table sets total (sqrt phase, exp
    phase) instead of thrashing Square/Ln/Exp loads
  - the output projection is interleaved per chunk with attention, with
    PSUM pools sized so attention + projection coexist in the 8 banks;
    bf16 rows stream out as soon as each 128-token tile is projected
"""

import numpy as np
import ml_dtypes

DIM, H, KV, S, B = 2048, 16, 4, 2048, 2
HD = DIM // H          # 128
GQ = H // KV           # 4 query heads per kv head
P = 128                # partitions
NK = DIM // P          # 16 contraction tiles
NCH = S // 512         # 4 sequence chunks of 512
EPS = 1e-6
BF = ml_dtypes.bfloat16

_CACHED = {}


def _build_program():
    import concourse.bass as bass
    import concourse.tile as tile
    from concourse import bacc
    from concourse import mybir
    from concourse.masks import make_identity

    f32 = mybir.dt.float32
    bf16 = mybir.dt.bfloat16
    AF = mybir.ActivationFunctionType

    nc = bacc.Bacc()
    xt4 = nc.declare_dram_parameter("xt4", [P, NCH, NK, 512], bf16, isOutput=False)
    wq = nc.declare_dram_parameter("wq", [P, GQ, NK, HD], bf16, isOutput=False)
    wk = nc.declare_dram_parameter("wk", [P, NK, HD], bf16, isOutput=False)
    wv = nc.declare_dram_parameter("wv", [P, NK, HD], bf16, isOutput=False)
    wo = nc.declare_dram_parameter("wo", [P, GQ, DIM], bf16, isOutput=False)
    cosq = nc.declare_dram_parameter("cosq", [HD, S], bf16, isOutput=False)
    sinq = nc.declare_dram_parameter("sinq", [HD, S], bf16, isOutput=False)
    cosk = nc.declare_dram_parameter("cosk", [HD, S], bf16, isOutput=False)
    sink = nc.declare_dram_parameter("sink", [HD, S], bf16, isOutput=False)
    mtri = nc.declare_dram_parameter("mtri", [P, 1280], bf16, isOutput=False)
    rsw = nc.declare_dram_parameter("rsw", [P, P], bf16, isOutput=False)
    po = nc.declare_dram_parameter("po", [S, DIM], bf16, isOutput=True)

    inv_sqrt_hd = 1.0 / float(np.sqrt(HD))

    with tile.TileContext(nc) as tc:
      with tc.tile_pool(name="const", bufs=1) as const, \
           tc.tile_pool(name="w5", bufs=1) as w5, \
           tc.tile_pool(name="hatp", bufs=1) as hatp:
        ones_sb = const.tile([P, P], bf16)
        nc.vector.memset(ones_sb, 1.0)
        osb = const.tile([P, P], bf16)           # 1/HD for the rmsnorm mean
        nc.vector.memset(osb, 1.0 / HD)
        ident = const.tile([P, P], bf16)
        make_identity(nc, ident)
        wmov = const.tile([P, 512], bf16)
        nc.vector.memset(wmov, 0.0)
        mtri_sb = const.tile([P, 1280], bf16)
        nc.gpsimd.dma_start(out=mtri_sb, in_=mtri[:, :])
        sq_dummy = const.tile([P, 1], f32)
        nc.vector.memset(sq_dummy, 1.0)
        sq_dummy_o = const.tile([P, 1], bf16)
        # preload the sqrt activation-table set during the DMA wait
        nc.scalar.activation(sq_dummy_o, sq_dummy, AF.Sqrt)

        wo_sb = w5.tile([P, GQ, DIM], bf16)
        nc.gpsimd.dma_start(out=wo_sb, in_=wo.ap().rearrange("p h n -> p (h n)"))

        v_nat = hatp.tile([P, NK, HD], bf16, tag="vnat")
        khat = hatp.tile([P, S], bf16, tag="khat")
        qhat = [hatp.tile([P, S], bf16, tag=f"qhat{h}", name=f"qhat{h}")
                for h in range(GQ)]
        onorm = [hatp.tile([P, S], bf16, tag=f"onorm{h}", name=f"onorm{h}")
                 for h in range(GQ)]

        # ---- warm-up: keep the PE busy while the first DMAs land ----
        with tc.tile_pool(name="wps", bufs=1, space="PSUM") as wps:
            wt = wps.tile([P, 512], f32, tag="warm")
            for _ in range(14):
                nc.tensor.matmul(wt, ones_sb, wmov, start=True, stop=True)

        # ---- phase A: projections + rmsnorm + rope, chunk-pipelined ----
        with tc.tile_pool(name="xtp", bufs=1) as xtp, \
             tc.tile_pool(name="xchk", bufs=2) as xchk, \
             tc.tile_pool(name="q32p", bufs=12) as q32p, \
             tc.tile_pool(name="vTp", bufs=2) as vTp, \
             tc.tile_pool(name="scr", bufs=2) as scr, \
             tc.tile_pool(name="psA", bufs=3, space="PSUM") as psA, \
             tc.tile_pool(name="psQ", bufs=2, space="PSUM") as psQ, \
             tc.tile_pool(name="psA2", bufs=2, space="PSUM") as psA2:
            # sync's DMA queue is several times slower than the scalar and
            # gpsimd queues, so everything latency-critical goes on those two
            wk_sb = xtp.tile([P, NK, HD], bf16, tag="wk")
            nc.scalar.dma_start(out=wk_sb, in_=wk.ap().rearrange("p j n -> p (j n)"))
            wq_sb = xtp.tile([P, GQ, NK, HD], bf16, tag="wq")
            wv_sb = xtp.tile([P, NK, HD], bf16, tag="wv")
            rsw_sb = xtp.tile([P, P], bf16, tag="rsw")
            cs_sb = {}
            for nm, t in (("cosq", cosq), ("sinq", sinq), ("cosk", cosk), ("sink", sink)):
                cs_sb[nm] = xtp.tile([P, S], bf16, tag=f"cs_{nm}", name=f"cs_{nm}")

            def dma_chunk(c):
                # four quarter-DMAs on the two fast queues so the k-slot
                # accumulation can begin as soon as its j-tiles land
                xt_c = xchk.tile([P, NK, 512], bf16, tag="xt", name=f"xt{c}")
                xsrc = xt4.ap()[:, c]
                for qn in range(4):
                    jsl = slice(qn * 4, (qn + 1) * 4)
                    eng = nc.scalar if qn % 2 == 0 else nc.gpsimd
                    eng.dma_start(out=xt_c[:, jsl, :], in_=xsrc[:, jsl])
                return xt_c

            xts = {0: dma_chunk(0)}
            nc.scalar.dma_start(out=wv_sb, in_=wv.ap().rearrange("p j n -> p (j n)"))
            for h in range(2):
                nc.scalar.dma_start(out=wq_sb[:, h], in_=wq.ap()[:, h])
            nc.gpsimd.dma_start(out=rsw_sb, in_=rsw[:, :])
            for nm, t in (("cosk", cosk), ("sink", sink)):
                nc.gpsimd.dma_start(out=cs_sb[nm], in_=t[:, :])
            xts[1] = dma_chunk(1)
            for h in range(2, GQ):
                nc.scalar.dma_start(out=wq_sb[:, h], in_=wq.ap()[:, h])
            for nm, t in (("cosq", cosq), ("sinq", sinq)):
                nc.gpsimd.dma_start(out=cs_sb[nm], in_=t[:, :])

            def p1(c):
                xt_c = xts.pop(c)
                srcs = {}
                for slot in (4, 5, 0, 1, 2, 3):
                    ps = psA.tile([P, 512], f32, tag="proj")
                    for j in range(NK):
                        if slot < 4:
                            lhs = wq_sb[:, slot, j, :]
                        elif slot == 4:
                            lhs = wk_sb[:, j, :]
                        else:
                            lhs = wv_sb[:, j, :]
                        nc.tensor.matmul(ps, lhs, xt_c[:, j, :],
                                         start=(j == 0), stop=(j == NK - 1))
                    if slot == 5:
                        vT_c = vTp.tile([P, 512], bf16, tag="vT")
                        nc.scalar.copy(vT_c, ps)
                        tp = psA2.tile([P, 512], bf16, tag="vtr", bufs=1)
                        for u in range(4):
                            nc.tensor.transpose(tp[:, u * HD:(u + 1) * HD],
                                                vT_c[:, u * HD:(u + 1) * HD], ident)
                        nc.scalar.copy(v_nat[:, 4 * c:4 * c + 4, :], tp)
                    else:
                        t32 = q32p.tile([P, 512], bf16, tag="q32",
                                        name=f"q32_{c}_{slot}")
                        nc.scalar.copy(t32, ps)
                        srcs[slot] = t32
                return srcs

            def p2(c, srcs):
                sl = slice(c * 512, (c + 1) * 512)
                for t in (4, 0, 1, 2, 3):
                    src = srcs[t]
                    dst = qhat[t] if t < 4 else khat
                    cosT = cs_sb["cosq" if t < 4 else "cosk"]
                    sinT = cs_sb["sinq" if t < 4 else "sink"]
                    sqb = scr.tile([P, 512], bf16, tag="sqb")
                    nc.scalar.activation(sqb, src, AF.Square)
                    ssq = psQ.tile([P, 512], f32, tag="ssq")
                    nc.tensor.matmul(ssq, osb, sqb, start=True, stop=True)
                    msinv = scr.tile([P, 512], f32, tag="msinv")
                    nc.vector.reciprocal_approx_fast(out=msinv, in_=ssq)
                    rsb = scr.tile([P, 512], bf16, tag="rsb")
                    nc.scalar.activation(rsb, msinv, AF.Sqrt)
                    rot = psA2.tile([P, 512], f32, tag="rot")
                    nc.tensor.matmul(rot, rsw_sb, src, start=True, stop=True)
                    rot_sb = scr.tile([P, 512], bf16, tag="rot_sb")
                    nc.scalar.copy(rot_sb, rot)
                    t1 = scr.tile([P, 512], bf16, tag="t1")
                    nc.vector.tensor_mul(t1, src, cosT[:, sl])
                    t2 = scr.tile([P, 512], bf16, tag="t2")
                    nc.vector.tensor_mul(t2, rot_sb, sinT[:, sl])
                    t3 = scr.tile([P, 512], bf16, tag="t3")
                    nc.vector.tensor_add(t3, t1, t2)
                    nc.vector.tensor_mul(dst[:, sl], t3, rsb)

            # software-pipelined: P2 for chunk c-1 is emitted after P1 for
            # chunk c, so its small matmuls never head-of-line block P1
            prev = None
            for c in range(NCH):
                if c + 1 < NCH and c + 1 not in xts:
                    xts[c + 1] = dma_chunk(c + 1)
                cur = (c, p1(c))
                if prev is not None:
                    p2(*prev)
                prev = cur
            p2(*prev)

        # ---- phase B: attention + output projection, per chunk ----
        with tc.tile_pool(name="ptp", bufs=34) as ptp, \
             tc.tile_pool(name="accp", bufs=4) as accp, \
             tc.tile_pool(name="recp", bufs=2) as recp, \
             tc.tile_pool(name="rowp", bufs=2) as rowp, \
             tc.tile_pool(name="psc", bufs=2, space="PSUM") as psc, \
             tc.tile_pool(name="pss", bufs=1, space="PSUM") as pss:
            # chunk 1 first: its inputs are ready long before the phase-A
            # tail finishes, and it carries enough tensor work to bridge
            # the A->B transition without the PE clock re-throttling
            for c in (1, 2, 0, 3):
                sl = slice(c * 512, (c + 1) * 512)
                # pass 1: scores -> exp -> mask -> DVE denominator adds for
                # all four heads; pass 2 (den matmul, P@V, normalize) follows
                # so the tensor queue always has ready work to pull
                pvs = {}
                accs = {}
                for h in range(GQ):
                    # off-diagonal key-tile pairs: full 512-query width
                    pv_list = []
                    for pr in range(2 * c):
                        sc = psc.tile([P, 1024], f32, tag="sc",
                                      name=f"sc_{c}_{h}_{pr}")
                        for u in range(2):
                            j = 2 * pr + u
                            nc.tensor.matmul(sc[:, u * 512:(u + 1) * 512],
                                             khat[:, j * P:(j + 1) * P],
                                             qhat[h][:, sl],
                                             start=True, stop=True)
                        pt = ptp.tile([P, 1024], bf16, tag="pt",
                                      name=f"pt_{c}_{h}_{pr}")
                        nc.scalar.activation(pt, sc, AF.Exp, scale=inv_sqrt_hd)
                        pv_list.append((2 * pr, pt, 0, 0, 512))
                        pv_list.append((2 * pr + 1, pt, 512, 0, 512))
                    # diagonal 512-block: restricted query ranges
                    # tile u covers queries [128u, 512) of the chunk
                    scA = psc.tile([P, 1024], f32, tag="sc", name=f"scA_{c}_{h}")
                    nc.tensor.matmul(scA[:, 0:512],
                                     khat[:, (4 * c) * P:(4 * c + 1) * P],
                                     qhat[h][:, c * 512:(c + 1) * 512],
                                     start=True, stop=True)
                    nc.tensor.matmul(scA[:, 512:896],
                                     khat[:, (4 * c + 1) * P:(4 * c + 2) * P],
                                     qhat[h][:, c * 512 + 128:(c + 1) * 512],
                                     start=True, stop=True)
                    ptA = ptp.tile([P, 1024], bf16, tag="pt", name=f"ptA_{c}_{h}")
                    nc.scalar.activation(ptA[:, 0:896], scA[:, 0:896],
                                         AF.Exp, scale=inv_sqrt_hd)
                    nc.vector.tensor_mul(ptA[:, 0:896], ptA[:, 0:896],
                                         mtri_sb[:, 0:896])
                    scB = psc.tile([P, 1024], f32, tag="sc", name=f"scB_{c}_{h}")
                    nc.tensor.matmul(scB[:, 0:256],
                                     khat[:, (4 * c + 2) * P:(4 * c + 3) * P],
                                     qhat[h][:, c * 512 + 256:(c + 1) * 512],
                                     start=True, stop=True)
                    nc.tensor.matmul(scB[:, 256:384],
                                     khat[:, (4 * c + 3) * P:(4 * c + 4) * P],
                                     qhat[h][:, c * 512 + 384:(c + 1) * 512],
                                     start=True, stop=True)
                    ptB = ptp.tile([P, 1024], bf16, tag="pt", name=f"ptB_{c}_{h}")
                    nc.scalar.activation(ptB[:, 0:384], scB[:, 0:384],
                                         AF.Exp, scale=inv_sqrt_hd)
                    nc.vector.tensor_mul(ptB[:, 0:384], ptB[:, 0:384],
                                         mtri_sb[:, 896:1280])
                    pv_list.append((4 * c + 0, ptA, 0, 0, 512))
                    pv_list.append((4 * c + 1, ptA, 512, 128, 384))
                    pv_list.append((4 * c + 2, ptB, 0, 256, 256))
                    pv_list.append((4 * c + 3, ptB, 256, 384, 128))
                    pvs[h] = pv_list

                    # denominator: accumulate probability tiles on DVE
                    acc = accp.tile([P, 512], bf16, tag="acc", name=f"acc_{c}_{h}")
                    first = True
                    for (_, pt, co, qo, w) in pv_list:
                        if first:
                            nc.vector.tensor_copy(acc, pt[:, co:co + w])
                            first = False
                        else:
                            nc.vector.tensor_add(acc[:, qo:qo + w],
                                                 acc[:, qo:qo + w],
                                                 pt[:, co:co + w])
                    accs[h] = acc

                # pass 2: partition-reduce den, P@V, normalize
                for h in range(GQ):
                    pv_list = pvs[h]
                    den = pss.tile([P, 512], f32, tag="den", name=f"den_{c}_{h}")
                    nc.tensor.matmul(den, ones_sb, accs[h], start=True, stop=True)
                    rec = recp.tile([P, 512], f32, tag="rec")
                    nc.vector.reciprocal_approx_fast(out=rec, in_=den)

                    ots = pss.tile([P, 512], f32, tag="ots", bufs=2,
                                   name=f"ot_{c}_{h}")
                    n_pv = len(pv_list)
                    for idx, (j, pt, co, qo, w) in enumerate(pv_list):
                        nc.tensor.matmul(ots[:, qo:qo + w], v_nat[:, j, :],
                                         pt[:, co:co + w],
                                         start=(idx == 0), stop=(idx == n_pv - 1))
                    nc.vector.tensor_mul(onorm[h][:, sl], ots, rec)

                # output projection for this chunk's 4 token tiles
                for i in range(4 * c, 4 * c + 4):
                    isl = slice(i * P, (i + 1) * P)
                    row = rowp.tile([P, DIM], bf16, tag="row", name=f"row_{i}")
                    for n in range(NCH):
                        po_ps = pss.tile([P, 512], f32, tag="po",
                                         name=f"po_{i}_{n}")
                        for h in range(GQ):
                            nc.tensor.matmul(po_ps, onorm[h][:, isl],
                                             wo_sb[:, h, n * 512:(n + 1) * 512],
                                             start=(h == 0), stop=(h == GQ - 1))
                        nc.vector.tensor_copy(row[:, n * 512:(n + 1) * 512], po_ps)
                    if c == NCH - 1:
                        # processed last: split across both queues so the
                        # final transfer is short
                        nc.scalar.dma_start(out=po[isl, 0:1024], in_=row[:, 0:1024])
                        nc.sync.dma_start(out=po[isl, 1024:2048], in_=row[:, 1024:2048])
                    else:
                        nc.sync.dma_start(out=po[isl, :], in_=row)
    nc.compile()
    return nc


def _causal_ok(mask):
    m = np.asarray(mask).reshape(S, S)
    tri = np.tril(np.ones((S, S), dtype=bool))
    return bool(np.all(m[tri] == 0.0) and np.all(m[~tri] <= -1e8))


def _reference_fallback(x, Wq, Wk, Wv, Wo, qg, kg, cos, sin, mask):
    x64 = np.asarray(x, dtype=np.float32)
    q = (x64 @ Wq).reshape(B, S, H, HD).transpose(0, 2, 1, 3)
    k = (x64 @ Wk).reshape(B, S, KV, HD).transpose(0, 2, 1, 3)
    v = (x64 @ Wv).reshape(B, S, KV, HD).transpose(0, 2, 1, 3)

    def rms(t, g):
        r = np.sqrt(np.mean(t * t, axis=-1, keepdims=True) + EPS)
        return g * (t / r)

    q, k = rms(q, qg), rms(k, kg)

    def rot(t):
        return np.concatenate([-t[..., HD // 2:], t[..., :HD // 2]], axis=-1)

    c = cos[None, None, :, :]
    s = sin[None, None, :, :]
    q = q * c + rot(q) * s
    k = k * c + rot(k) * s
    k = np.repeat(k, GQ, axis=1)
    v = np.repeat(v, GQ, axis=1)
    sc = np.einsum('bhqd,bhkd->bhqk', q, k) / np.sqrt(HD) + np.asarray(mask).reshape(1, 1, S, S)
    sc = sc - sc.max(axis=-1, keepdims=True)
    e = np.exp(sc)
    a = e / e.sum(axis=-1, keepdims=True)
    o = np.einsum('bhqk,bhkd->bhqd', a, v)
    o = o.transpose(0, 2, 1, 3).reshape(B, S, H * HD)
    return (o @ Wo).astype(np.float32)


def kernel(x, Wq, Wk, Wv, Wo, qg, kg, cos, sin, mask, **_unused):
    x = np.asarray(x, dtype=np.float32)
    Wq, Wk, Wv, Wo = (np.asarray(a, dtype=np.float32) for a in (Wq, Wk, Wv, Wo))
    qg, kg = np.asarray(qg, np.float32), np.asarray(kg, np.float32)
    cos, sin = np.asarray(cos, np.float32), np.asarray(sin, np.float32)
    if not _causal_ok(mask):
        return _reference_fallback(x, Wq, Wk, Wv, Wo, qg, kg, cos, sin, mask)

    from concourse.bass_utils import run_bass_kernel_spmd

    if "nc" not in _CACHED:
        _CACHED["nc"] = _build_program()
    nc = _CACHED["nc"]

    cosT = np.ascontiguousarray(cos.T)  # [HD, S]
    sinT = np.ascontiguousarray(sin.T)

    # rope via halves: out[:64] = x[:64]*cos[:64] + x[64:]*sin_tbl[:64]
    #                  out[64:] = x[64:]*cos[64:] + x[:64]*sin_tbl[64:]
    # reference: rot(x)[:64] = -x[64:], rot(x)[64:] = x[:64]; gains fold in.
    def tables(g):
        ct = cosT * g[:, None]
        st = np.empty_like(sinT)
        st[:64] = -sinT[:64] * g[64:, None]
        st[64:] = sinT[64:] * g[:64, None]
        return ct.astype(BF), st.astype(BF)

    cq, sq = tables(qg)
    ck, sk = tables(kg)

    rsw = np.zeros((P, P), dtype=np.float32)
    for i in range(P):
        rsw[i, (i + 64) % P] = 1.0
    rsw = rsw.astype(BF)

    # restricted-diagonal masks: within each 128-column sub-range that
    # starts a diagonal tile, query-col >= key-row; elsewhere 1.
    rows = np.arange(P)[:, None]
    tri = (np.arange(P)[None, :] >= rows)          # [128,128] step
    onesP = np.ones((P, P), dtype=bool)
    mA = np.concatenate([tri, onesP, onesP, onesP, tri, onesP, onesP], axis=1)  # 896
    mB = np.concatenate([tri, onesP, tri], axis=1)                              # 384
    mtri = np.concatenate([mA, mB], axis=1).astype(BF)                          # [128,1280]

    def part_layout(w, cols):
        # [DIM, cols] -> [P, NK, cols] with feature d = j*128 + p
        return np.ascontiguousarray(w.reshape(NK, P, cols).transpose(1, 0, 2)).astype(BF)

    xt4 = []
    for b in range(B):
        xT = x[b].T  # [DIM, S]
        xt4.append(np.ascontiguousarray(
            xT.reshape(NK, P, NCH, 512).transpose(1, 2, 0, 3)).astype(BF))

    in_maps = []
    for core in range(8):
        b, g = divmod(core, KV)
        wo_g = Wo[g * GQ * HD:(g + 1) * GQ * HD, :]
        wq_g = Wq[:, g * GQ * HD:(g + 1) * GQ * HD]  # [DIM, 4*HD]
        in_maps.append({
            "xt4": xt4[b],
            # head-major [P, GQ, NK, HD] so each head's slice is one
            # contiguous per-partition DMA
            "wq": np.ascontiguousarray(
                wq_g.reshape(NK, P, GQ, HD).transpose(1, 2, 0, 3)).astype(BF),
            "wk": part_layout(Wk[:, g * HD:(g + 1) * HD], HD),
            "wv": part_layout(Wv[:, g * HD:(g + 1) * HD], HD),
            "wo": np.ascontiguousarray(
                wo_g.reshape(GQ, P, DIM).transpose(1, 0, 2)).astype(BF),
            "cosq": cq, "sinq": sq, "cosk": ck, "sink": sk,
            "mtri": mtri, "rsw": rsw,
        })

    res = run_bass_kernel_spmd(nc, in_maps, list(range(8)))
    out = np.zeros((B, S, DIM), dtype=np.float32)
    for core in range(8):
        out[core // KV] += res.results[core]["po"].astype(np.float32)
    return out


# revision 26
# speedup vs baseline: 1.1886x; 1.1886x over previous
"""GroupedQueryAttention Trainium2 kernel (8 NeuronCores).

Sharding: (batch b in 0..1) x (kv-head group g in 0..3) -> core 4*b+g.
Each core computes, for its batch, the 4 query heads (4g..4g+3) that share
kv head g, plus the partial output projection through the matching 512-row
slice of Wo.  The host sums the 4 bf16 partials per batch in f32.

On-device dataflow is fully "transposed": activations live as [feature,
token] so every matmul contraction sits on the partition axis, and the
softmax probabilities come out directly in the layout the P@V matmul
needs.  Performance structure vs the phase-serial baseline:
  - host pre-arranges every DRAM operand so each DMA is contiguous per
    partition; x arrives in per-chunk pieces on the sync queue while the
    weight tensors stream on the scalar queue
  - dummy warm-up matmuls run during the initial DMA wait so the PE HAM
    clock gate is at 8/8 when real work starts
  - causal diagonal 512-blocks are computed on restricted query ranges
    (512/384/256/128 wide) instead of full width + mask
  - softmax denominators are accumulated on the Vector engine (bf16 adds
    over the probability tiles) with a single ones-matmul partition
    reduction per (chunk, head); 1/den uses reciprocal_approx_fast
  - rmsnorm uses reciprocal_approx_fast + a Sqrt activation, keeping the
    Scalar engine on two activation-table sets total (sqrt phase, exp
    phase) instead of thrashing Square/Ln/Exp loads
  - the output projection is interleaved per chunk with attention, with
    PSUM pools sized so attention + projection coexist in the 8 banks;
    bf16 rows stream out as soon as each 128-token tile is projected
"""

import numpy as np
import ml_dtypes

DIM, H, KV, S, B = 2048, 16, 4, 2048, 2
HD = DIM // H          # 128
GQ = H // KV           # 4 query heads per kv head
P = 128                # partitions
NK = DIM // P          # 16 contraction tiles
NCH = S // 512         # 4 sequence chunks of 512
EPS = 1e-6
BF = ml_dtypes.bfloat16

_CACHED = {}


def _build_program():
    import concourse.bass as bass
    import concourse.tile as tile
    from concourse import bacc
    from concourse import mybir
    from concourse.masks import make_identity

    f32 = mybir.dt.float32
    bf16 = mybir.dt.bfloat16
    AF = mybir.ActivationFunctionType

    nc = bacc.Bacc()
    xt4 = nc.declare_dram_parameter("xt4", [P, NCH, NK, 512], bf16, isOutput=False)
    wq = nc.declare_dram_parameter("wq", [P, GQ, NK, HD], bf16, isOutput=False)
    wk = nc.declare_dram_parameter("wk", [P, NK, HD], bf16, isOutput=False)
    wv = nc.declare_dram_parameter("wv", [P, NK, HD], bf16, isOutput=False)
    wo = nc.declare_dram_parameter("wo", [P, GQ, DIM], bf16, isOutput=False)
    cosq = nc.declare_dram_parameter("cosq", [HD, S], bf16, isOutput=False)
    sinq = nc.declare_dram_parameter("sinq", [HD, S], bf16, isOutput=False)
    cosk = nc.declare_dram_parameter("cosk", [HD, S], bf16, isOutput=False)
    sink = nc.declare_dram_parameter("sink", [HD, S], bf16, isOutput=False)
    mtri = nc.declare_dram_parameter("mtri", [P, 1280], bf16, isOutput=False)
    rsw = nc.declare_dram_parameter("rsw", [P, P], bf16, isOutput=False)
    po = nc.declare_dram_parameter("po", [S, DIM], bf16, isOutput=True)

    inv_sqrt_hd = 1.0 / float(np.sqrt(HD))

    with tile.TileContext(nc) as tc:
      with tc.tile_pool(name="const", bufs=1) as const, \
           tc.tile_pool(name="w5", bufs=1) as w5, \
           tc.tile_pool(name="hatp", bufs=1) as hatp:
        ones_sb = const.tile([P, P], bf16)
        nc.vector.memset(ones_sb, 1.0)
        osb = const.tile([P, P], bf16)           # 1/HD for the rmsnorm mean
        nc.vector.memset(osb, 1.0 / HD)
        ident = const.tile([P, P], bf16)
        make_identity(nc, ident)
        wmov = const.tile([P, 512], bf16)
        nc.vector.memset(wmov, 0.0)
        mtri_sb = const.tile([P, 1280], bf16)
        nc.gpsimd.dma_start(out=mtri_sb, in_=mtri[:, :])
        sq_dummy = const.tile([P, 1], f32)
        nc.vector.memset(sq_dummy, 1.0)
        sq_dummy_o = const.tile([P, 1], bf16)
        # preload the sqrt activation-table set during the DMA wait
        nc.scalar.activation(sq_dummy_o, sq_dummy, AF.Sqrt)

        wo_sb = w5.tile([P, GQ, DIM], bf16)
        nc.gpsimd.dma_start(out=wo_sb, in_=wo.ap().rearrange("p h n -> p (h n)"))

        v_nat = hatp.tile([P, NK, HD], bf16, tag="vnat")
        khat = hatp.tile([P, S], bf16, tag="khat")
        qhat = [hatp.tile([P, S], bf16, tag=f"qhat{h}", name=f"qhat{h}")
                for h in range(GQ)]
        onorm = [hatp.tile([P, S], bf16, tag=f"onorm{h}", name=f"onorm{h}")
                 for h in range(GQ)]

        # ---- warm-up: keep the PE busy while the first DMAs land ----
        with tc.tile_pool(name="wps", bufs=1, space="PSUM") as wps:
            wt = wps.tile([P, 512], f32, tag="warm")
            for _ in range(14):
                nc.tensor.matmul(wt, ones_sb, wmov, start=True, stop=True)

        # ---- phase A: projections + rmsnorm + rope, chunk-pipelined ----
        with tc.tile_pool(name="xtp", bufs=1) as xtp, \
             tc.tile_pool(name="xchk", bufs=2) as xchk, \
             tc.tile_pool(name="q32p", bufs=12) as q32p, \
             tc.tile_pool(name="vTp", bufs=2) as vTp, \
             tc.tile_pool(name="scr", bufs=2) as scr, \
             tc.tile_pool(name="psA", bufs=3, space="PSUM") as psA, \
             tc.tile_pool(name="psQ", bufs=2, space="PSUM") as psQ, \
             tc.tile_pool(name="psA2", bufs=2, space="PSUM") as psA2:
            # sync's DMA queue is several times slower than the scalar and
            # gpsimd queues, so everything latency-critical goes on those two
            wk_sb = xtp.tile([P, NK, HD], bf16, tag="wk")
            nc.scalar.dma_start(out=wk_sb, in_=wk.ap().rearrange("p j n -> p (j n)"))
            wq_sb = xtp.tile([P, GQ, NK, HD], bf16, tag="wq")
            wv_sb = xtp.tile([P, NK, HD], bf16, tag="wv")
            rsw_sb = xtp.tile([P, P], bf16, tag="rsw")
            cs_sb = {}
            for nm, t in (("cosq", cosq), ("sinq", sinq), ("cosk", cosk), ("sink", sink)):
                cs_sb[nm] = xtp.tile([P, S], bf16, tag=f"cs_{nm}", name=f"cs_{nm}")

            def dma_chunk(c):
                # four quarter-DMAs on the two fast queues so the k-slot
                # accumulation can begin as soon as its j-tiles land
                xt_c = xchk.tile([P, NK, 512], bf16, tag="xt", name=f"xt{c}")
                xsrc = xt4.ap()[:, c]
                for qn in range(4):
                    jsl = slice(qn * 4, (qn + 1) * 4)
                    eng = nc.scalar if qn % 2 == 0 else nc.gpsimd
                    eng.dma_start(out=xt_c[:, jsl, :], in_=xsrc[:, jsl])
                return xt_c

            xts = {0: dma_chunk(0)}
            nc.scalar.dma_start(out=wv_sb, in_=wv.ap().rearrange("p j n -> p (j n)"))
            for h in range(2):
                nc.scalar.dma_start(out=wq_sb[:, h], in_=wq.ap()[:, h])
            nc.gpsimd.dma_start(out=rsw_sb, in_=rsw[:, :])
            for nm, t in (("cosk", cosk), ("sink", sink)):
                nc.gpsimd.dma_start(out=cs_sb[nm], in_=t[:, :])
            xts[1] = dma_chunk(1)
            for h in range(2, GQ):
                nc.scalar.dma_start(out=wq_sb[:, h], in_=wq.ap()[:, h])
            for nm, t in (("cosq", cosq), ("sinq", sinq)):
                nc.gpsimd.dma_start(out=cs_sb[nm], in_=t[:, :])

            def p1(c):
                xt_c = xts.pop(c)
                srcs = {}
                for slot in (4, 5, 0, 1, 2, 3):
                    ps = psA.tile([P, 512], f32, tag="proj")
                    for j in range(NK):
                        if slot < 4:
                            lhs = wq_sb[:, slot, j, :]
                        elif slot == 4:
                            lhs = wk_sb[:, j, :]
                        else:
                            lhs = wv_sb[:, j, :]
                        nc.tensor.matmul(ps, lhs, xt_c[:, j, :],
                                         start=(j == 0), stop=(j == NK - 1))
                    if slot == 5:
                        vT_c = vTp.tile([P, 512], bf16, tag="vT")
                        nc.scalar.copy(vT_c, ps)
                        tp = psA2.tile([P, 512], bf16, tag="vtr", bufs=1)
                        for u in range(4):
                            nc.tensor.transpose(tp[:, u * HD:(u + 1) * HD],
                                                vT_c[:, u * HD:(u + 1) * HD], ident)
                        nc.scalar.copy(v_nat[:, 4 * c:4 * c + 4, :], tp)
                    else:
                        t32 = q32p.tile([P, 512], bf16, tag="q32",
                                        name=f"q32_{c}_{slot}")
                        nc.scalar.copy(t32, ps)
                        srcs[slot] = t32
                return srcs

            def p2(c, srcs):
                sl = slice(c * 512, (c + 1) * 512)
                for t in (4, 0, 1, 2, 3):
                    src = srcs[t]
                    dst = qhat[t] if t < 4 else khat
                    cosT = cs_sb["cosq" if t < 4 else "cosk"]
                    sinT = cs_sb["sinq" if t < 4 else "sink"]
                    sqb = scr.tile([P, 512], bf16, tag="sqb")
                    nc.scalar.activation(sqb, src, AF.Square)
                    ssq = psQ.tile([P, 512], f32, tag="ssq")
                    nc.tensor.matmul(ssq, osb, sqb, start=True, stop=True)
                    msinv = scr.tile([P, 512], f32, tag="msinv")
                    nc.vector.reciprocal_approx_fast(out=msinv, in_=ssq)
                    rsb = scr.tile([P, 512], bf16, tag="rsb")
                    nc.scalar.activation(rsb, msinv, AF.Sqrt)
                    rot = psA2.tile([P, 512], f32, tag="rot")
                    nc.tensor.matmul(rot, rsw_sb, src, start=True, stop=True)
                    rot_sb = scr.tile([P, 512], bf16, tag="rot_sb")
                    nc.scalar.copy(rot_sb, rot)
                    t1 = scr.tile([P, 512], bf16, tag="t1")
                    nc.vector.tensor_mul(t1, src, cosT[:, sl])
                    t2 = scr.tile([P, 512], bf16, tag="t2")
                    nc.vector.tensor_mul(t2, rot_sb, sinT[:, sl])
                    t3 = scr.tile([P, 512], bf16, tag="t3")
                    nc.vector.tensor_add(t3, t1, t2)
                    nc.vector.tensor_mul(dst[:, sl], t3, rsb)

            # software-pipelined: P2 for chunk c-1 is emitted after P1 for
            # chunk c, so its small matmuls never head-of-line block P1
            prev = None
            for c in range(NCH):
                if c + 1 < NCH and c + 1 not in xts:
                    xts[c + 1] = dma_chunk(c + 1)
                cur = (c, p1(c))
                if prev is not None:
                    p2(*prev)
                prev = cur
            p2(*prev)

        # ---- phase B: attention + output projection, per chunk ----
        with tc.tile_pool(name="ptp", bufs=34) as ptp, \
             tc.tile_pool(name="accp", bufs=4) as accp, \
             tc.tile_pool(name="recp", bufs=2) as recp, \
             tc.tile_pool(name="rowp", bufs=2) as rowp, \
             tc.tile_pool(name="psc", bufs=2, space="PSUM") as psc, \
             tc.tile_pool(name="pss", bufs=1, space="PSUM") as pss:
            for c in range(NCH):
                sl = slice(c * 512, (c + 1) * 512)
                # pass 1: scores -> exp -> mask -> DVE denominator adds for
                # all four heads; pass 2 (den matmul, P@V, normalize) follows
                # so the tensor queue always has ready work to pull
                pvs = {}
                accs = {}
                for h in range(GQ):
                    # off-diagonal key-tile pairs: full 512-query width
                    pv_list = []
                    for pr in range(2 * c):
                        sc = psc.tile([P, 1024], f32, tag="sc",
                                      name=f"sc_{c}_{h}_{pr}")
                        for u in range(2):
                            j = 2 * pr + u
                            nc.tensor.matmul(sc[:, u * 512:(u + 1) * 512],
                                             khat[:, j * P:(j + 1) * P],
                                             qhat[h][:, sl],
                                             start=True, stop=True)
                        pt = ptp.tile([P, 1024], bf16, tag="pt",
                                      name=f"pt_{c}_{h}_{pr}")
                        nc.scalar.activation(pt, sc, AF.Exp, scale=inv_sqrt_hd)
                        pv_list.append((2 * pr, pt, 0, 0, 512))
                        pv_list.append((2 * pr + 1, pt, 512, 0, 512))
                    # diagonal 512-block: restricted query ranges
                    # tile u covers queries [128u, 512) of the chunk
                    scA = psc.tile([P, 1024], f32, tag="sc", name=f"scA_{c}_{h}")
                    nc.tensor.matmul(scA[:, 0:512],
                                     khat[:, (4 * c) * P:(4 * c + 1) * P],
                                     qhat[h][:, c * 512:(c + 1) * 512],
                                     start=True, stop=True)
                    nc.tensor.matmul(scA[:, 512:896],
                                     khat[:, (4 * c + 1) * P:(4 * c + 2) * P],
                                     qhat[h][:, c * 512 + 128:(c + 1) * 512],
                                     start=True, stop=True)
                    ptA = ptp.tile([P, 1024], bf16, tag="pt", name=f"ptA_{c}_{h}")
                    nc.scalar.activation(ptA[:, 0:896], scA[:, 0:896],
                                         AF.Exp, scale=inv_sqrt_hd)
                    nc.vector.tensor_mul(ptA[:, 0:896], ptA[:, 0:896],
                                         mtri_sb[:, 0:896])
                    scB = psc.tile([P, 1024], f32, tag="sc", name=f"scB_{c}_{h}")
                    nc.tensor.matmul(scB[:, 0:256],
                                     khat[:, (4 * c + 2) * P:(4 * c + 3) * P],
                                     qhat[h][:, c * 512 + 256:(c + 1) * 512],
                                     start=True, stop=True)
                    nc.tensor.matmul(scB[:, 256:384],
                                     khat[:, (4 * c + 3) * P:(4 * c + 4) * P],
                                     qhat[h][:, c * 512 + 384:(c + 1) * 512],
                                     start=True, stop=True)
                    ptB = ptp.tile([P, 1024], bf16, tag="pt", name=f"ptB_{c}_{h}")
                    nc.scalar.activation(ptB[:, 0:384], scB[:, 0:384],
                                         AF.Exp, scale=inv_sqrt_hd)
                    nc.vector.tensor_mul(ptB[:, 0:384], ptB[:, 0:384],
                                         mtri_sb[:, 896:1280])
                    pv_list.append((4 * c + 0, ptA, 0, 0, 512))
                    pv_list.append((4 * c + 1, ptA, 512, 128, 384))
                    pv_list.append((4 * c + 2, ptB, 0, 256, 256))
                    pv_list.append((4 * c + 3, ptB, 256, 384, 128))
                    pvs[h] = pv_list

                    # denominator: accumulate probability tiles on DVE
                    acc = accp.tile([P, 512], bf16, tag="acc", name=f"acc_{c}_{h}")
                    first = True
                    for (_, pt, co, qo, w) in pv_list:
                        if first:
                            nc.vector.tensor_copy(acc, pt[:, co:co + w])
                            first = False
                        else:
                            nc.vector.tensor_add(acc[:, qo:qo + w],
                                                 acc[:, qo:qo + w],
                                                 pt[:, co:co + w])
                    accs[h] = acc

                # pass 2: partition-reduce den, P@V, normalize
                for h in range(GQ):
                    pv_list = pvs[h]
                    den = pss.tile([P, 512], f32, tag="s", bufs=4,
                                   name=f"den_{c}_{h}")
                    nc.tensor.matmul(den, ones_sb, accs[h], start=True, stop=True)
                    rec = recp.tile([P, 512], f32, tag="rec")
                    nc.vector.reciprocal_approx_fast(out=rec, in_=den)

                    ots = pss.tile([P, 512], f32, tag="s", bufs=4,
                                   name=f"ot_{c}_{h}")
                    n_pv = len(pv_list)
                    for idx, (j, pt, co, qo, w) in enumerate(pv_list):
                        nc.tensor.matmul(ots[:, qo:qo + w], v_nat[:, j, :],
                                         pt[:, co:co + w],
                                         start=(idx == 0), stop=(idx == n_pv - 1))
                    nc.vector.tensor_mul(onorm[h][:, sl], ots, rec)

                # output projection for this chunk's 4 token tiles
                for i in range(4 * c, 4 * c + 4):
                    isl = slice(i * P, (i + 1) * P)
                    row = rowp.tile([P, DIM], bf16, tag="row", name=f"row_{i}")
                    for n in range(NCH):
                        po_ps = pss.tile([P, 512], f32, tag="s", bufs=4,
                                         name=f"po_{i}_{n}")
                        for h in range(GQ):
                            nc.tensor.matmul(po_ps, onorm[h][:, isl],
                                             wo_sb[:, h, n * 512:(n + 1) * 512],
                                             start=(h == 0), stop=(h == GQ - 1))
                        nc.vector.tensor_copy(row[:, n * 512:(n + 1) * 512], po_ps)
                    if c == NCH - 1:
                        # processed last: split across both queues so the
                        # final transfer is short
                        nc.scalar.dma_start(out=po[isl, 0:1024], in_=row[:, 0:1024])
                        nc.sync.dma_start(out=po[isl, 1024:2048], in_=row[:, 1024:2048])
                    else:
                        nc.sync.dma_start(out=po[isl, :], in_=row)
    nc.compile()
    return nc


def _causal_ok(mask):
    m = np.asarray(mask).reshape(S, S)
    tri = np.tril(np.ones((S, S), dtype=bool))
    return bool(np.all(m[tri] == 0.0) and np.all(m[~tri] <= -1e8))


def _reference_fallback(x, Wq, Wk, Wv, Wo, qg, kg, cos, sin, mask):
    x64 = np.asarray(x, dtype=np.float32)
    q = (x64 @ Wq).reshape(B, S, H, HD).transpose(0, 2, 1, 3)
    k = (x64 @ Wk).reshape(B, S, KV, HD).transpose(0, 2, 1, 3)
    v = (x64 @ Wv).reshape(B, S, KV, HD).transpose(0, 2, 1, 3)

    def rms(t, g):
        r = np.sqrt(np.mean(t * t, axis=-1, keepdims=True) + EPS)
        return g * (t / r)

    q, k = rms(q, qg), rms(k, kg)

    def rot(t):
        return np.concatenate([-t[..., HD // 2:], t[..., :HD // 2]], axis=-1)

    c = cos[None, None, :, :]
    s = sin[None, None, :, :]
    q = q * c + rot(q) * s
    k = k * c + rot(k) * s
    k = np.repeat(k, GQ, axis=1)
    v = np.repeat(v, GQ, axis=1)
    sc = np.einsum('bhqd,bhkd->bhqk', q, k) / np.sqrt(HD) + np.asarray(mask).reshape(1, 1, S, S)
    sc = sc - sc.max(axis=-1, keepdims=True)
    e = np.exp(sc)
    a = e / e.sum(axis=-1, keepdims=True)
    o = np.einsum('bhqk,bhkd->bhqd', a, v)
    o = o.transpose(0, 2, 1, 3).reshape(B, S, H * HD)
    return (o @ Wo).astype(np.float32)


def kernel(x, Wq, Wk, Wv, Wo, qg, kg, cos, sin, mask, **_unused):
    x = np.asarray(x, dtype=np.float32)
    Wq, Wk, Wv, Wo = (np.asarray(a, dtype=np.float32) for a in (Wq, Wk, Wv, Wo))
    qg, kg = np.asarray(qg, np.float32), np.asarray(kg, np.float32)
    cos, sin = np.asarray(cos, np.float32), np.asarray(sin, np.float32)
    if not _causal_ok(mask):
        return _reference_fallback(x, Wq, Wk, Wv, Wo, qg, kg, cos, sin, mask)

    from concourse.bass_utils import run_bass_kernel_spmd

    if "nc" not in _CACHED:
        _CACHED["nc"] = _build_program()
    nc = _CACHED["nc"]

    cosT = np.ascontiguousarray(cos.T)  # [HD, S]
    sinT = np.ascontiguousarray(sin.T)

    # rope via halves: out[:64] = x[:64]*cos[:64] + x[64:]*sin_tbl[:64]
    #                  out[64:] = x[64:]*cos[64:] + x[:64]*sin_tbl[64:]
    # reference: rot(x)[:64] = -x[64:], rot(x)[64:] = x[:64]; gains fold in.
    def tables(g):
        ct = cosT * g[:, None]
        st = np.empty_like(sinT)
        st[:64] = -sinT[:64] * g[64:, None]
        st[64:] = sinT[64:] * g[:64, None]
        return ct.astype(BF), st.astype(BF)

    cq, sq = tables(qg)
    ck, sk = tables(kg)

    rsw = np.zeros((P, P), dtype=np.float32)
    for i in range(P):
        rsw[i, (i + 64) % P] = 1.0
    rsw = rsw.astype(BF)

    # restricted-diagonal masks: within each 128-column sub-range that
    # starts a diagonal tile, query-col >= key-row; elsewhere 1.
    rows = np.arange(P)[:, None]
    tri = (np.arange(P)[None, :] >= rows)          # [128,128] step
    onesP = np.ones((P, P), dtype=bool)
    mA = np.concatenate([tri, onesP, onesP, onesP, tri, onesP, onesP], axis=1)  # 896
    mB = np.concatenate([tri, onesP, tri], axis=1)                              # 384
    mtri = np.concatenate([mA, mB], axis=1).astype(BF)                          # [128,1280]

    def part_layout(w, cols):
        # [DIM, cols] -> [P, NK, cols] with feature d = j*128 + p
        return np.ascontiguousarray(w.reshape(NK, P, cols).transpose(1, 0, 2)).astype(BF)

    xt4 = []
    for b in range(B):
        xT = x[b].T  # [DIM, S]
        xt4.append(np.ascontiguousarray(
            xT.reshape(NK, P, NCH, 512).transpose(1, 2, 0, 3)).astype(BF))

    in_maps = []
    for core in range(8):
        b, g = divmod(core, KV)
        wo_g = Wo[g * GQ * HD:(g + 1) * GQ * HD, :]
        wq_g = Wq[:, g * GQ * HD:(g + 1) * GQ * HD]  # [DIM, 4*HD]
        in_maps.append({
            "xt4": xt4[b],
            # head-major [P, GQ, NK, HD] so each head's slice is one
            # contiguous per-partition DMA
            "wq": np.ascontiguousarray(
                wq_g.reshape(NK, P, GQ, HD).transpose(1, 2, 0, 3)).astype(BF),
            "wk": part_layout(Wk[:, g * HD:(g + 1) * HD], HD),
            "wv": part_layout(Wv[:, g * HD:(g + 1) * HD], HD),
            "wo": np.ascontiguousarray(
                wo_g.reshape(GQ, P, DIM).transpose(1, 0, 2)).astype(BF),
            "cosq": cq, "sinq": sq, "cosk": ck, "sink": sk,
            "mtri": mtri, "rsw": rsw,
        })

    res = run_bass_kernel_spmd(nc, in_maps, list(range(8)))
    out = np.zeros((B, S, DIM), dtype=np.float32)
    for core in range(8):
        out[core // KV] += res.results[core]["po"].astype(np.float32)
    return out
